# revision 2
# baseline (speedup 1.0000x reference)
"""LRMC (masked low-rank matrix completion) kernel for 8 trn2 NeuronCores.

Algorithm (3 iterations):
    E = mask * (x - U V)        [d1, d2]
    U = U + E V^T               [d1, r]
    V = V + U^T E               [r, d2]   (updated U; V-update all-reduced)
out = U V.

Sharding: rows of x/mask/U over 8 cores (512 rows each); V replicated;
U^T E contribution all-reduced (bf16) each iteration.

On-chip layouts (per core, m = 512 local rows, r = 128, d2 = 4096):
    mxT, maskT : [128p, 32nb, 512m]  bf16   (transposed masked-x / mask)
    e_T tiles  : [128p(d2), 512m]    bf16   (rotating)
    e_nat      : [128p(m), 4mb, 4096d2] bf16  (via DMA xbar transpose)
    U stored as U_T f32/bf16 [128r, 512m] + U_nat bf16 [128p(m), 4mb, 128r]
    V stored as V_nat bf16 [128r, 4096d2] + V_T bf16 [128p(d2), 32nb, 128r]

Matmuls (all bf16 operands, fp32 PSUM):
    P_T[nb]   = matmul(lhsT=V_nat[:, nb*128:+128], rhs=U_T)          (M1)
    Udelta_T += matmul(lhsT=V_T[:, nb], rhs=e_T[nb])                 (M2)
    G[chunk] += matmul(lhsT=U_nat[:, mb], rhs=e_nat[:, mb, chunk])   (M3)
"""

import numpy as np
import ml_dtypes

import concourse.bass as bass
import concourse.tile as tile
from concourse import bacc, mybir
from concourse.bass_utils import run_bass_kernel_spmd

D1, D2, RANK, ITERS = 4096, 4096, 128, 3
N_CORES = 8
M = D1 // N_CORES          # 512 rows per core
MB = M // 128              # 4 row blocks
NB = D2 // 128             # 32 d2 blocks
NCHUNK = D2 // 512         # 8 d2 chunks of 512

BF16 = mybir.dt.bfloat16
F32 = mybir.dt.float32
bf16 = ml_dtypes.bfloat16

_cache = {}


def _build():
    nc = bacc.Bacc("TRN2", target_bir_lowering=False, debug=False,
                   num_devices=N_CORES)

    mxT_d = nc.dram_tensor("mxT", [128, NB, M], BF16, kind="ExternalInput")
    mskT_d = nc.dram_tensor("mskT", [128, NB, M], BF16, kind="ExternalInput")
    uT32_d = nc.dram_tensor("uT32", [128, M], F32, kind="ExternalInput")
    unat_d = nc.dram_tensor("unat", [128, MB, 128], BF16, kind="ExternalInput")
    vb_d = nc.dram_tensor("vb", [128, D2], BF16, kind="ExternalInput")
    vT_d = nc.dram_tensor("vT", [128, NB, 128], BF16, kind="ExternalInput")
    out_d = nc.dram_tensor("out", [M, D2], F32, kind="ExternalOutput")

    with tile.TileContext(nc) as tc:
        with (
            tc.tile_pool(name="state", bufs=1) as state,
            tc.tile_pool(name="ustate", bufs=2) as ustate,
            tc.tile_pool(name="enat", bufs=1) as enatp,
            tc.tile_pool(name="rot", bufs=3) as rot,
            tc.tile_pool(name="gout", bufs=2) as goutp,
            tc.tile_pool(name="ostage", bufs=2) as ostage,
            tc.tile_pool(name="psP", bufs=3, space="PSUM") as psP,
            tc.tile_pool(name="psU", bufs=1, space="PSUM") as psU,
            tc.tile_pool(name="psG", bufs=2, space="PSUM") as psG,
            tc.tile_pool(name="dram", bufs=2, space="DRAM") as dram,
        ):
            # ---- load state (small first so matmuls can start early)
            uT32 = state.tile([128, M], F32)
            uTb = ustate.tile([128, M], BF16, tag="uTb")
            unat = ustate.tile([128, MB, 128], BF16, tag="unat")
            vb = state.tile([128, D2], BF16)
            vT = state.tile([128, NB, 128], BF16)
            nc.sync.dma_start(uT32[:], uT32_d[:])
            nc.scalar.copy(uTb[:], uT32[:])
            nc.sync.dma_start(unat[:], unat_d[:])
            nc.sync.dma_start(vb[:], vb_d[:])
            nc.sync.dma_start(vT[:], vT_d[:])

            mxT = state.tile([128, NB, M], BF16)
            mskT = state.tile([128, NB, M], BF16)
            for c in range(4):
                s = slice(c * (NB // 4), (c + 1) * (NB // 4))
                nc.sync.dma_start(mxT[:, s], mxT_d[:, s])
                nc.sync.dma_start(mskT[:, s], mskT_d[:, s])

            for it in range(ITERS):
                e_nat = enatp.tile([128, MB, D2], BF16, tag="e_nat")
                ud_ps = psU.tile([128, M], F32, tag="ud")

                for nb in range(NB):
                    pT_ps = psP.tile([128, M], F32, tag="pT")
                    nc.tensor.matmul(
                        pT_ps[:], vb[:, nb * 128:(nb + 1) * 128], uTb[:],
                        start=True, stop=True,
                    )
                    pTb = rot.tile([128, M], BF16, tag="pTb")
                    nc.scalar.copy(pTb[:], pT_ps[:])
                    q = rot.tile([128, M], BF16, tag="q")
                    nc.vector.tensor_tensor(
                        q[:], mskT[:, nb], pTb[:], mybir.AluOpType.mult)
                    eT = rot.tile([128, M], BF16, tag="eT")
                    nc.vector.tensor_tensor(
                        eT[:], mxT[:, nb], q[:], mybir.AluOpType.subtract)
                    nc.tensor.matmul(
                        ud_ps[:], vT[:, nb], eT[:],
                        start=(nb == 0), stop=(nb == NB - 1),
                        skip_group_check=True,
                    )
                    nc.sync.dma_start_transpose(
                        e_nat[:, :, nb * 128:(nb + 1) * 128], eT[:])

                # ---- U update (U_T += Udelta_T), refresh bf16 + U_nat
                nc.vector.tensor_tensor(
                    uT32[:], uT32[:], ud_ps[:], mybir.AluOpType.add)
                uTb = ustate.tile([128, M], BF16, tag="uTb")
                nc.scalar.copy(uTb[:], uT32[:])
                unat = ustate.tile([128, MB, 128], BF16, tag="unat")
                nc.sync.dma_start_transpose(unat[:], uTb[:])

                # ---- M3: G = U_new^T E  (local contribution), then AllReduce
                cc_in = dram.tile([128, D2], BF16, tag="cc_in")
                cc_out = dram.tile([128, D2], BF16, addr_space="Shared",
                                   tag="cc_out")
                g_sb = goutp.tile([128, D2], BF16, tag="g")
                for ch in range(NCHUNK):
                    g_ps = psG.tile([128, 512], F32, tag="g")
                    for mb in range(MB):
                        nc.tensor.matmul(
                            g_ps[:], unat[:, mb],
                            e_nat[:, mb, ch * 512:(ch + 1) * 512],
                            start=(mb == 0), stop=(mb == MB - 1),
                        )
                    nc.scalar.copy(g_sb[:, ch * 512:(ch + 1) * 512], g_ps[:])
                    nc.sync.dma_start(
                        cc_in[:, ch * 512:(ch + 1) * 512],
                        g_sb[:, ch * 512:(ch + 1) * 512])

                nc.gpsimd.collective_compute(
                    "AllReduce",
                    mybir.AluOpType.add,
                    replica_groups=[list(range(N_CORES))],
                    ins=[cc_in.opt()],
                    outs=[cc_out.opt()],
                )

                # ---- V += dV  (natural and transposed copies)
                dv = goutp.tile([128, D2], BF16, tag="dv")
                nc.sync.dma_start(dv[:], cc_out[:])
                nc.vector.tensor_tensor(
                    vb[:], vb[:], dv[:], mybir.AluOpType.add)
                dvT = goutp.tile([128, NB, 128], BF16, tag="dvT")
                nc.sync.dma_start_transpose(dvT[:], cc_out[:])
                nc.vector.tensor_tensor(
                    vT[:], vT[:], dvT[:], mybir.AluOpType.add)

            # ---- output: P = U V in natural orientation, fp32
            for mb in range(MB):
                o_sb = ostage.tile([128, D2], F32, tag="o")
                for ch in range(NCHUNK):
                    o_ps = psP.tile([128, 512], F32, tag="pT")
                    nc.tensor.matmul(
                        o_ps[:], uTb[:, mb * 128:(mb + 1) * 128],
                        vb[:, ch * 512:(ch + 1) * 512],
                        start=True, stop=True,
                    )
                    eng = nc.scalar if ch % 2 == 0 else nc.vector
                    if ch % 2 == 0:
                        nc.scalar.copy(o_sb[:, ch * 512:(ch + 1) * 512], o_ps[:])
                    else:
                        nc.vector.tensor_copy(
                            o_sb[:, ch * 512:(ch + 1) * 512], o_ps[:])
                nc.sync.dma_start(
                    out_d[mb * 128:(mb + 1) * 128, :], o_sb[:])

    nc.compile()
    return nc


def _prep_inputs(x, mask, U, V):
    x = np.ascontiguousarray(np.asarray(x, dtype=np.float32))
    mask = np.ascontiguousarray(np.asarray(mask, dtype=np.float32))
    U = np.ascontiguousarray(np.asarray(U, dtype=np.float32))
    V = np.ascontiguousarray(np.asarray(V, dtype=np.float32))
    mx = mask * x

    vb = V.astype(bf16)                                    # [128, D2]
    vT = np.ascontiguousarray(
        V.T.reshape(NB, 128, 128).transpose(1, 0, 2)).astype(bf16)

    in_maps = []
    for i in range(N_CORES):
        rows = slice(i * M, (i + 1) * M)
        mxT = np.ascontiguousarray(
            mx[rows].T.reshape(NB, 128, M).transpose(1, 0, 2)).astype(bf16)
        mskT = np.ascontiguousarray(
            mask[rows].T.reshape(NB, 128, M).transpose(1, 0, 2)).astype(bf16)
        uT32 = np.ascontiguousarray(U[rows].T)             # [128, M] f32
        unat = np.ascontiguousarray(
            U[rows].reshape(MB, 128, 128).transpose(1, 0, 2)).astype(bf16)
        in_maps.append({
            "mxT": mxT, "mskT": mskT, "uT32": uT32, "unat": unat,
            "vb": vb, "vT": vT,
        })
    return in_maps


def kernel(x, mask, U, V, _trace=False):
    if "nc" not in _cache:
        _cache["nc"] = _build()
    nc = _cache["nc"]
    in_maps = _prep_inputs(x, mask, U, V)
    res = run_bass_kernel_spmd(
        nc, in_maps, core_ids=list(range(N_CORES)), trace=_trace)
    _cache["last_result"] = res
    out = np.concatenate([res.results[i]["out"] for i in range(N_CORES)],
                         axis=0)
    return out.astype(np.float32)
